# revision 34
# baseline (speedup 1.0000x reference)
"""TRN2 Bass/Tile kernel: Llama attention block (B=1, S=2048, D=2048, H=16, causal).

Sharding: tensor-parallel over heads. 16 heads / 8 cores = 2 heads per core.
Wq/Wk/Wv column-sharded (256 dims per core); output projection is
sequence-parallel after a per-head AllToAll of attention outputs.

v2 layout (all matmuls bf16 operands, fp32 PSUM accumulate):
  - qk projection runs kt-OUTER over 8 PSUM banks so the PE consumes xt
    tiles at the rate the HBM DMA delivers them (no startup starvation)
  - RoPE rotate-half matmuls interleave with the v-projection on the PE;
    the elementwise chain is spread over Pool (t1, fin) and DVE (t2)
  - attention scores land in [P, 1024] two-bank PSUM tiles so each ScalarE
    exp covers two t-tiles (halves the per-instruction access overhead)
  - softmax denominators: DVE binary-tree accumulation of the exp tiles
    (bf16, 4x mode) + one short ones-matmul per sq block, instead of a
    full-rate ones-matmul per tile
  - Wo is streamed from HBM kt-outer in [P, 2048] slices on the gpsimd DMA
    queue (no 8MB SBUF preload); pass A (head-0 k-slices) interleaves into
    head-1's attention gaps; output rows DMA directly from PSUM
"""

import os
import sys

import numpy as np

for _p in ("/opt/trn_rl_repo",):
    if _p not in sys.path and os.path.isdir(_p):
        sys.path.insert(0, _p)

P = 128            # SBUF partitions
S = 2048           # sequence length
D = 2048           # hidden dim
NCORES = 8
DC = D // NCORES   # 256 = head-dims per core
HPC = 2            # heads per core
HD = 128           # head dim
KT = D // P        # 16 contraction tiles
SQW = 512          # sq tile width (moving free dim)
NSQ = S // SQW     # 4
NT = S // P        # 16 t tiles
SCS = S // NCORES  # 256 output seq rows per core (sequence-parallel Wo)
SM = float(1.0 / np.sqrt(HD))

_NC_CACHE = {}
LAST_RESULTS = None


def _build_nc(reps=1):
    import concourse.bacc as bacc
    import concourse.mybir as mybir
    from concourse import tile

    fp32 = mybir.dt.float32
    bf16 = mybir.dt.bfloat16
    Exp = mybir.ActivationFunctionType.Exp

    nc = bacc.Bacc("TRN2", num_devices=NCORES, debug=False)

    xt = nc.dram_tensor("xt", [D, S], bf16, kind="ExternalInput")
    wq = nc.dram_tensor("wq", [D, DC], bf16, kind="ExternalInput")
    wk = nc.dram_tensor("wk", [D, DC], bf16, kind="ExternalInput")
    wv = nc.dram_tensor("wv", [D, DC], bf16, kind="ExternalInput")
    wo = nc.dram_tensor("wo", [D, D], bf16, kind="ExternalInput")  # full Wo.T
    cost = nc.dram_tensor("cost", [HD, S], bf16, kind="ExternalInput")
    sint = nc.dram_tensor("sint", [HD, S], bf16, kind="ExternalInput")
    rt = nc.dram_tensor("rt", [HD, HD], bf16, kind="ExternalInput")
    msk = nc.dram_tensor("msk", [P, 2, 2 * SQW], bf16, kind="ExternalInput")
    ones = nc.dram_tensor("ones", [P, P], bf16, kind="ExternalInput")
    out = nc.dram_tensor("out", [SCS, D], fp32, kind="ExternalOutput")

    xt_r = xt.rearrange("(k p) s -> p k s", p=P)
    wq_r = wq.rearrange("(k p) d -> p k d", p=P)
    wk_r = wk.rearrange("(k p) d -> p k d", p=P)
    wv_r = wv.rearrange("(k p) d -> p k d", p=P)
    wo_r = wo.rearrange("(k p) d -> p k d", p=P)
    out_r = out.rearrange("(m p) d -> m p d", p=P)

    def emit_body(tc, rep):
        r = f"r{rep}"
        with (
            tc.tile_pool(name=f"const{r}", bufs=1) as const,
            tc.tile_pool(name=f"acts{r}", bufs=1) as acts,
            tc.tile_pool(name=f"work{r}", bufs=2) as work,
            tc.tile_pool(name=f"ps{r}", bufs=2, space="PSUM") as ps,
            tc.tile_pool(name=f"dram{r}", bufs=1, space="DRAM") as dram,
        ):
            # ---- constants / weights ----
            wq_sb = const.tile([P, KT, DC], bf16, name=f"wq_sb{r}")
            wk_sb = const.tile([P, KT, DC], bf16, name=f"wk_sb{r}")
            wv_sb = const.tile([P, KT, DC], bf16, name=f"wv_sb{r}")
            cos_sb = const.tile([HD, S], bf16, name=f"cos_sb{r}")
            sin_sb = const.tile([HD, S], bf16, name=f"sin_sb{r}")
            rt_sb = const.tile([HD, HD], bf16, name=f"rt_sb{r}")
            msk_sb = const.tile([P, 2, 2 * SQW], bf16, name=f"msk_sb{r}")
            ones_sb = const.tile([P, P], bf16, name=f"ones_sb{r}")
            # q/k weights on the scalar queue: ready before the first xt
            # tile; head-0 halves first so the first matmul starts sooner
            nc.scalar.dma_start(wq_sb[:, 0, 0:HD], wq_r[:, 0, 0:HD])
            nc.scalar.dma_start(wq_sb[:, 1:KT, 0:HD], wq_r[:, 1:KT, 0:HD])
            nc.scalar.dma_start(wk_sb[:, :, 0:HD], wk_r[:, :, 0:HD])
            nc.scalar.dma_start(wq_sb[:, :, HD:DC], wq_r[:, :, HD:DC])
            nc.scalar.dma_start(wk_sb[:, :, HD:DC], wk_r[:, :, HD:DC])
            # everything needed later follows on the scalar queue
            nc.scalar.dma_start(rt_sb[:], rt[:])
            nc.scalar.dma_start(cos_sb[:], cost[:])
            nc.scalar.dma_start(sin_sb[:], sint[:])
            nc.scalar.dma_start(wv_sb[:], wv_r)
            nc.scalar.dma_start(msk_sb[:], msk[:])
            nc.scalar.dma_start(ones_sb[:], ones[:])

            # ---- persistent activations ----
            qfin = acts.tile([HD, HPC, S], bf16, name=f"qfin{r}")
            kfin = acts.tile([HD, HPC, S], bf16, name=f"kfin{r}")
            v_sb = acts.tile([P, NT, DC], bf16, name=f"v_sb{r}")
            attnT = acts.tile([HD, HPC, S], bf16, name=f"attnT{r}")
            # h-major so each head's gather is one contiguous DMA
            ag_sb = acts.tile([P, HPC, NCORES, SCS], bf16, name=f"ag_sb{r}")

            a2a_in = [dram.tile([NCORES, HD, SCS], bf16, name=f"a2ain{h}{r}")
                      for h in range(HPC)]
            a2a_out = [dram.tile([NCORES, HD, SCS], bf16, name=f"a2aout{h}{r}")
                       for h in range(HPC)]

            # PSUM tags (8 banks total): two [P,1024] two-bank wide tiles
            # (pscA/pscB) + two [P,512] double-buffered tags (pa/pl).
            def wide(name):
                return ps.tile([P, 2 * SQW], fp32, tag=name, bufs=1, name=name)

            def narrow(tag, name):
                return ps.tile([P, SQW], fp32, tag=tag, bufs=2, name=name)

            # ---- attention helpers ----
            # Per head h, per sq block i: npj = 2i+2 wide j-pairs. Scores for
            # pair pj (t tiles 2pj, 2pj+1) land in a wide psum tile; one exp
            # covers both halves; DVE tree-sums the (masked) exp tiles into
            # the denominator; av matmuls consume the halves.
            def tree_push(stack, tile_):
                h_ = 0
                while stack and stack[-1][0] == h_:
                    _, prev = stack.pop()
                    s_ = work.tile([P, 2 * SQW], bf16, tag="acc",
                                   bufs=6, name="acc")
                    nc.vector.tensor_add(s_[:], prev[:], tile_[:])
                    tile_ = s_
                    h_ += 1
                stack.append((h_, tile_))

            def attn_pair(h, i, pj, npj, pa_ap, psc, stack, start, stop):
                sq = slice(i * SQW, (i + 1) * SQW)
                for half in range(2):
                    j = 2 * pj + half
                    nc.tensor.matmul(
                        psc[:, half * SQW:(half + 1) * SQW],
                        kfin[:, h, j * P:(j + 1) * P],
                        qfin[:, h, sq],
                        start=True, stop=True, skip_group_check=True,
                    )
                e = work.tile([P, 2 * SQW], bf16, tag="e", bufs=4, name="e")
                nc.scalar.activation(e[:], psc[:], Exp, scale=SM)
                if pj >= npj - 2:
                    # diagonal region: mask pair p (j tiles 4i+2p, 4i+2p+1)
                    pp = pj - (npj - 2)
                    em = work.tile([P, 2 * SQW], bf16, tag="em", bufs=2,
                                   name="em")
                    nc.vector.tensor_mul(em[:], e[:], msk_sb[:, pp, :])
                    e = em
                for half in range(2):
                    j = 2 * pj + half
                    nc.tensor.matmul(
                        pa_ap,
                        v_sb[:, j, h * HD:(h + 1) * HD],
                        e[:, half * SQW:(half + 1) * SQW],
                        start=(start and half == 0),
                        stop=(stop and half == 1),
                        skip_group_check=True,
                    )
                tree_push(stack, e)

            def attn_flush(h, i, pa_ap, stack, pl_ap=None):
                if pl_ap is None:
                    pl_ap = narrow("pl", "pl")[:]
                tiles = [t for _, t in stack]
                for t, tile_ in enumerate(tiles):
                    for half in range(2):
                        nc.tensor.matmul(
                            pl_ap,
                            ones_sb[:],
                            tile_[:, half * SQW:(half + 1) * SQW],
                            start=(t == 0 and half == 0),
                            stop=(t == len(tiles) - 1 and half == 1),
                            skip_group_check=True,
                        )
                sq = slice(i * SQW, (i + 1) * SQW)
                rec = work.tile([P, SQW], fp32, tag="rec", bufs=2, name="rec")
                nc.vector.reciprocal(rec[:], pl_ap)
                nc.vector.tensor_mul(attnT[:, h, sq], pa_ap, rec[:])
                st = None
                for jj in (2 * i, 2 * i + 1):
                    st = nc.sync.dma_start(
                        a2a_in[h][jj, :, :],
                        attnT[:, h, jj * SCS:(jj + 1) * SCS],
                    )
                return st

            def emit_attention(h, blocks):
                deferred = [None]  # tail work of the previous i block
                last_store = [None]

                def flush_tail():
                    if deferred[0] is None:
                        return
                    i, pa, stack = deferred[0]
                    deferred[0] = None
                    last_store[0] = attn_flush(h, i, pa[:], stack)

                for i in blocks:
                    npj = 2 * i + 2
                    pa = narrow("pa", "pa")
                    stack = []
                    for pj in range(npj):
                        psc = wide("pscA" if pj % 2 == 0 else "pscB")
                        attn_pair(h, i, pj, npj, pa[:], psc, stack,
                                  start=(pj == 0), stop=(pj == npj - 1))
                        if pj == 1:
                            flush_tail()  # previous block's softmax + stores
                    deferred[0] = (i, pa, stack)
                flush_tail()
                return last_store[0]

            with tc.tile_pool(name=f"xtp{r}", bufs=1) as xtp:
                xt_sb = xtp.tile([P, KT, S], bf16, name=f"xt_sb{r}")
                # rope is applied in place: projections land in qfin/kfin,
                # then fin = raw*cos + rot(raw)*sin overwrites each slice
                qraw, kraw = qfin, kfin
                for kt in range(KT):
                    nc.sync.dma_start(xt_sb[:, kt, :], xt_r[:, kt, :])

                # ---- qk projection: kt-outer over 8 banks per head-pass ----
                for m in range(HPC):
                    msl = slice(m * HD, (m + 1) * HD)
                    qA, qB = wide(f"pscA"), wide(f"pscB")
                    kps = [narrow("pa", "kp0"), narrow("pa", "kp1"),
                           narrow("pl", "kp2"), narrow("pl", "kp3")]
                    for kt in range(KT):
                        st, sp = (kt == 0), (kt == KT - 1)
                        for half in range(2):
                            nc.tensor.matmul(
                                qA[:, half * SQW:(half + 1) * SQW],
                                wq_sb[:, kt, msl],
                                xt_sb[:, kt, half * SQW:(half + 1) * SQW],
                                start=st, stop=sp, skip_group_check=True,
                            )
                            nc.tensor.matmul(
                                qB[:, half * SQW:(half + 1) * SQW],
                                wq_sb[:, kt, msl],
                                xt_sb[:, kt, (2 + half) * SQW:(3 + half) * SQW],
                                start=st, stop=sp, skip_group_check=True,
                            )
                        for n in range(NSQ):
                            nc.tensor.matmul(
                                kps[n][:],
                                wk_sb[:, kt, msl],
                                xt_sb[:, kt, n * SQW:(n + 1) * SQW],
                                start=st, stop=sp, skip_group_check=True,
                            )
                    nc.scalar.copy(qraw[:, m, 0:2 * SQW], qA[:])
                    nc.scalar.copy(qraw[:, m, 2 * SQW:4 * SQW], qB[:])
                    for n in range(NSQ):
                        nc.scalar.copy(
                            kraw[:, m, n * SQW:(n + 1) * SQW], kps[n][:])

                # ---- rope (PE part interleaved with v projection) ----
                # k-m0 and q-m0 first so the hoisted h0 block can start early
                rope_slices = [(raw, fin, m, n)
                               for m in range(HPC)
                               for raw, fin, ns in ((kraw, kfin, (0, 1, 2, 3)),
                                                    (qraw, qfin, (3, 2, 1, 0)))
                               for n in ns]

                def emit_rope(s):
                    raw, fin, m, n = rope_slices[s]
                    nsl = slice(n * SQW, (n + 1) * SQW)
                    pr = narrow("pa" if s % 2 == 0 else "pl", "pr")
                    nc.tensor.matmul(pr[:], rt_sb[:], raw[:, m, nsl],
                                     start=True, stop=True)
                    t1 = work.tile([P, SQW], bf16, tag="t1", bufs=3, name="t1")
                    t2 = work.tile([P, SQW], bf16, tag="t2", bufs=3, name="t2")
                    nc.vector.tensor_mul(t1[:], raw[:, m, nsl], cos_sb[:, nsl])
                    nc.vector.tensor_mul(t2[:], pr[:], sin_sb[:, nsl])
                    nc.vector.tensor_add(fin[:, m, nsl], t1[:], t2[:])

                # ---- v projection: four m-tiles per wide psum group ----
                for p4 in range(NT // 4):
                    pv = wide("pscA" if p4 % 2 == 0 else "pscB")
                    emit_rope(4 * p4)
                    emit_rope(4 * p4 + 1)
                    for q in range(4):
                        mt = 4 * p4 + q
                        for kt in range(KT):
                            nc.tensor.matmul(
                                pv[:, q * DC:(q + 1) * DC],
                                xt_sb[:, kt, mt * P:(mt + 1) * P],
                                wv_sb[:, kt, :],
                                start=(kt == 0), stop=(kt == KT - 1),
                                skip_group_check=True,
                            )
                    emit_rope(4 * p4 + 2)
                    emit_rope(4 * p4 + 3)
                    nc.scalar.copy(v_sb[:, 4 * p4:4 * p4 + 4, :], pv[:])

            import bass_rust as _br
            emit_attention(0, blocks=[0, 1, 2, 3])

            # exchange head 0 while head 1 computes
            nc.gpsimd.collective_compute(
                "AllToAll",
                mybir.AluOpType.bypass,
                replica_groups=[list(range(NCORES))],
                ins=[a2a_in[0][:].opt()],
                outs=[a2a_out[0][:].opt()],
            )
            wo_t = [None] * KT

            def load_wo(kt):
                wt = work.tile([P, D], bf16, tag="wo", bufs=4, name="wo_t")
                nc.sync.dma_start(wt[:], wo_r[:, kt, :])
                wo_t[kt] = wt

            h1_store = emit_attention(1, blocks=[0, 1, 2, 3])

            # keep the c0-gated gather loads behind the last h1 store so the
            # scheduler cannot head-of-line-block the stores on any queue
            for j in range(NCORES):
                d = nc.scalar.dma_start(ag_sb[:, 0, j, :], a2a_out[0][j, :, :])
                _br.add_dep_helper(d.ins, h1_store.ins, False,
                                   "ag0 after h1 stores")
            # pass-A wo slices prefetch on the scalar queue
            for ki in range(NCORES):
                load_wo(2 * ki)

            nc.gpsimd.collective_compute(
                "AllToAll",
                mybir.AluOpType.bypass,
                replica_groups=[list(range(NCORES))],
                ins=[a2a_in[1][:].opt()],
                outs=[a2a_out[1][:].opt()],
            )
            for j in range(NCORES):
                d = nc.scalar.dma_start(ag_sb[:, 1, j, :], a2a_out[1][j, :, :])
                _br.add_dep_helper(d.ins, h1_store.ins, False,
                                   "ag1 after h1 stores")

            # ---- output projection (pass A overlaps the second AllToAll) ----
            po_m0 = [wide("pscA"), wide("pscB")]
            po_m1 = [narrow("pa", "po2"), narrow("pa", "po3"),
                     narrow("pl", "po4"), narrow("pl", "po5")]

            def wo_mms(h, ki):
                kt = 2 * ki + h
                st = (h == 0 and ki == 0)
                sp = (h == HPC - 1 and ki == NCORES - 1)
                for m in range(2):
                    for n in range(NSQ):
                        dst = (po_m0[n // 2][:, (n % 2) * SQW:(n % 2 + 1) * SQW]
                               if m == 0 else po_m1[n][:])
                        nc.tensor.matmul(
                            dst,
                            ag_sb[:, h, ki, m * P:(m + 1) * P],
                            wo_t[kt][:, n * SQW:(n + 1) * SQW],
                            start=st, stop=sp, skip_group_check=True,
                        )

            def wo_mms_one(h, ki, m, n):
                kt = 2 * ki + h
                dst = (po_m0[n // 2][:, (n % 2) * SQW:(n % 2 + 1) * SQW]
                       if m == 0 else po_m1[n][:])
                nc.tensor.matmul(
                    dst,
                    ag_sb[:, h, ki, m * P:(m + 1) * P],
                    wo_t[kt][:, n * SQW:(n + 1) * SQW],
                    start=(h == 0 and ki == 0),
                    stop=(h == 1 and ki == NCORES - 1),
                    skip_group_check=True,
                )

            for ki in range(NCORES):
                wo_mms(0, ki)
            for ki in range(NCORES):
                load_wo(2 * ki + 1)
            for ki in range(4):
                wo_mms(1, ki)
            # final 4 k-slices group-by-group so each output chunk drains
            # (copy + DMA) while the remaining groups still accumulate
            def drain(m, n):
                ob = work.tile([P, SQW], fp32, tag="ob", bufs=4, name="ob")
                if m == 0:
                    nc.scalar.copy(
                        ob[:],
                        po_m0[n // 2][:, (n % 2) * SQW:(n % 2 + 1) * SQW])
                else:
                    nc.vector.tensor_copy(ob[:], po_m1[n][:])
                nc.sync.dma_start(out_r[m][:, n * SQW:(n + 1) * SQW],
                                  ob[:])

            for m in range(2):
                for n in range(NSQ):
                    for ki in range(4, NCORES):
                        wo_mms_one(1, ki, m, n)
                    drain(m, n)

    with tile.TileContext(nc) as tc:
        for rep in range(reps):
            emit_body(tc, rep)

    nc.compile()
    return nc


def _get_nc(reps=1):
    key = ("nc", reps)
    if key not in _NC_CACHE:
        _NC_CACHE[key] = _build_nc(reps)
    return _NC_CACHE[key]


def _host_tables():
    import ml_dtypes

    bf = ml_dtypes.bfloat16
    inv_freq = 1.0 / (10000.0 ** (np.arange(0, HD, 2, dtype=np.float32) / HD))
    t = np.arange(S, dtype=np.float32)
    freqs = np.outer(t, inv_freq)
    emb = np.concatenate([freqs, freqs], axis=-1)        # [S, HD]
    cosT = np.ascontiguousarray(np.cos(emb).T).astype(bf)
    sinT = np.ascontiguousarray(np.sin(emb).T).astype(bf)

    rt = np.zeros((HD, HD), dtype=np.float32)
    for e in range(64):
        rt[e, e + 64] = 1.0
    for e in range(64, HD):
        rt[e, e - 64] = -1.0

    # mask pair p covers diagonal j-tiles m=2p, 2p+1 (t offsets within the
    # sq block): msk(m)[t, s] = (t <= s - 128*m) for s in [0, 512)
    y = np.arange(SQW)[None, :]
    tl = np.arange(P)[:, None]
    mskP = np.zeros((P, 2, 2 * SQW), dtype=np.float32)
    for pp in range(2):
        for half in range(2):
            m = 2 * pp + half
            mskP[:, pp, half * SQW:(half + 1) * SQW] = (
                tl <= (y - 128 * m)).astype(np.float32)

    onesM = np.ones((P, P), dtype=np.float32)
    return cosT, sinT, rt.astype(bf), mskP.astype(bf), onesM.astype(bf)


def _prep_in_maps(hidden_states, Wq, Wk, Wv, Wo):
    import ml_dtypes

    bf = ml_dtypes.bfloat16
    X = np.asarray(hidden_states, dtype=np.float32).reshape(S, D)
    Wq = np.asarray(Wq, dtype=np.float32)
    Wk = np.asarray(Wk, dtype=np.float32)
    Wv = np.asarray(Wv, dtype=np.float32)
    Wo = np.asarray(Wo, dtype=np.float32)

    XT = np.ascontiguousarray(X.T).astype(bf)
    WoT = np.ascontiguousarray(Wo.T).astype(bf)
    cosT, sinT, rt, mskP, onesM = _host_tables()

    in_maps = []
    for c in range(NCORES):
        sl = slice(DC * c, DC * (c + 1))
        in_maps.append({
            "xt": XT,
            "wq": np.ascontiguousarray(Wq[sl].T).astype(bf),
            "wk": np.ascontiguousarray(Wk[sl].T).astype(bf),
            "wv": np.ascontiguousarray(Wv[sl].T).astype(bf),
            "wo": WoT,
            "cost": cosT,
            "sint": sinT,
            "rt": rt,
            "msk": mskP,
            "ones": onesM,
        })
    return in_maps


def kernel(hidden_states, Wq, Wk, Wv, Wo):
    global LAST_RESULTS
    from concourse.bass_utils import run_bass_kernel_spmd

    in_maps = _prep_in_maps(hidden_states, Wq, Wk, Wv, Wo)
    nc = _get_nc()
    res = run_bass_kernel_spmd(nc, in_maps, core_ids=list(range(NCORES)))
    LAST_RESULTS = res

    out = np.concatenate(
        [np.asarray(res.results[c]["out"]) for c in range(NCORES)], axis=0
    )
    return out.reshape(1, S, D).astype(np.float32)
